# revision 8
# baseline (speedup 1.0000x reference)
"""Trainium2 Bass kernel for the Aligner2 problem (v2: fp8 DoubleRow).

Computes, for each batch b:
  k = LReLU(conv3(LReLU(conv3(keys))))                   # [256, 520]
  q = LReLU(conv7(LReLU(conv7(LReLU(conv7(queries))))))  # [256, 2048]
  sp[t,s] = sum_c q[c,t] k[c,s] - 0.5*k2[s]              # PE
  l = 2*TEMP*sp  (the -TEMP*q2 term cancels in log_softmax)
  attn = exp(l)/z;  logp = l - ln(z);  z = sum_s exp(l)

Precision: query convs in fp8e4m3 with DoubleRow (two 128-deep
contraction subtiles per pass), key convs and scores in f16, the
-0.5*k2 rank-1 row as an fp8 hi/lo DoubleRow pair, outputs f16
(upcast to f32 on host).

Engine split (GpSimd cannot touch PSUM on TRN2): Exp/lnz and the
qc1/qc2 leaky-ReLUs on Scalar (bias via the act bias port); kc1/kc2/qc3
leaky-ReLUs plus z/rz/logp on DVE (their biases enter the PSUM group as
rank-1 bias x ones matmuls); ksq / attn scaling on GpSimd (SBUF only).
Data-parallel over batch: 4 batches/core, 8 cores.
"""
import numpy as np

import concourse.bass as bass
import concourse.bacc as bacc
import concourse.tile as tile
from concourse import mybir
from concourse.bass_utils import run_bass_kernel_spmd

F32 = mybir.dt.float32
F16 = mybir.dt.float16
DT8 = mybir.dt.float8e4
AF = mybir.ActivationFunctionType
ALU = mybir.AluOpType
DR = mybir.MatmulPerfMode.DoubleRow

SLOPE = 0.3
TEMPERATURE = 0.0005
SC = 2.0 * TEMPERATURE

BPC = 4          # batches per core
N_CORES = 8
D_DEC, TQ = 80, 2048
D_ENC, TK = 512, 512
DH = 256
TK1 = TK + 4     # 516 after key conv1 (kernel 3, pad 3)
TK2 = TK + 8     # 520 after key conv2
HT1 = TK1 // 2   # 258
HT2 = TK2 // 2   # 260

import os
def _flag(name, default):
    return os.environ.get(name, default) == "1"

# per-layer matmul precision: fp8e4m3 DoubleRow vs plain f16
KC1_FP8 = _flag("KV2_KC1", "0")
KC2_FP8 = _flag("KV2_KC2", "0")
QC1_FP8 = _flag("KV2_QC1", "1")
QC2_FP8 = _flag("KV2_QC2", "1")
QC3_FP8 = _flag("KV2_QC3", "1")
K2_DR = _flag("KV2_K2DR", "1")      # fp8 hi/lo DoubleRow k2 row vs f16 single
GPS_OPS = _flag("KV2_GPS", "1")     # gpsimd for ksq/k2lo/attn + logp dma
OUT_GROUP = _flag("KV2_OUTG", "1")  # grouped strided out-DMA vs per-tile
DVE3_PRELU = _flag("KV2_DVE3", "1")  # qc3 prelu on DVE (else Scalar)

# remove back-to-back LDWEIGHTS with identical weight APs before compile
DEDUP_LDW = _flag("KV2_DEDUP", "0")

ACT_SET_ALL = 6  # act_info.json set containing Prelu/Exp/Ln/Copy together

DT_K = DT8 if KC1_FP8 else F16    # kpad, kw1t
DT_K1 = DT8 if KC2_FP8 else F16   # k1pad, kw2t
DT_Q = DT8 if QC1_FP8 else F16    # qpad, qw1t
DT_Q1 = DT8 if QC2_FP8 else F16   # q1pad, qw2t
DT_Q2 = DT8 if QC3_FP8 else F16   # q2pad, qw3t


def build_program():
    nc = bacc.Bacc("TRN2", target_bir_lowering=False)

    # two planes: plane 1 is pre-shifted left by one sample on the host so
    # tap-pair DoubleRow reads are plain strided slices (overlapping APs
    # crash walrus codegen)
    q_in = nc.dram_tensor("queries", [BPC, D_DEC, 2, TQ + 8], DT_Q, kind="ExternalInput")
    k_in = nc.dram_tensor("keys", [BPC, D_ENC, TK + 6], DT_K, kind="ExternalInput")
    kw1t_d = nc.dram_tensor("kw1t", [4, 128, 3, DH], DT_K, kind="ExternalInput")
    kw2t_d = nc.dram_tensor("kw2t", [2, 128, 3, DH], DT_K1, kind="ExternalInput")
    qw1t_d = nc.dram_tensor("qw1t", [D_DEC, 8, DH], DT_Q, kind="ExternalInput")
    qw2t_d = nc.dram_tensor("qw2t", [2, 128, 7, DH], DT_Q1, kind="ExternalInput")
    qw3t_d = nc.dram_tensor("qw3t", [2, 128, 7, DH], DT_Q2, kind="ExternalInput")
    # row-format biases feed rank-1 bias matmuls (DVE-prelu layers);
    # col-format biases feed the act bias port (Scalar-prelu layers)
    kb1_d = nc.dram_tensor("kb1c", [2, 128, 1], F32, kind="ExternalInput")
    kb2_d = nc.dram_tensor("kb2c", [2, 128, 1], F32, kind="ExternalInput")
    qb1_d = nc.dram_tensor("qb1c", [2, 128, 1], F32, kind="ExternalInput")
    qb2_d = nc.dram_tensor("qb2c", [2, 128, 1], F32, kind="ExternalInput")
    qb3_d = nc.dram_tensor("qb3c", [2, 128, 1], F32, kind="ExternalInput")
    attn_out = nc.dram_tensor("attn_out", [BPC, TQ, TK2], F16, kind="ExternalOutput")
    logp_out = nc.dram_tensor("logp_out", [BPC, TQ, TK2], F16, kind="ExternalOutput")

    with tile.TileContext(nc) as tc:
        _emit(nc, tc, q_in, k_in, kw1t_d, kw2t_d, qw1t_d, qw2t_d, qw3t_d,
              kb1_d, kb2_d, qb1_d, qb2_d, qb3_d, attn_out, logp_out)
    if DEDUP_LDW:
        _dedup_ldweights(nc)
    nc.compile()
    return nc


def _dedup_ldweights(nc):
    """Drop an InstLdweights whose weights AP is identical to the previous
    InstLdweights in the same block (nothing between them invalidates the
    PE-resident weights). Its waits/updates move to the following matmul;
    multi-wait legalization runs later in compile."""
    for bb in nc.main_func.blocks:
        insts = bb.instructions
        last_key = None
        drop = []
        for idx, inst in enumerate(insts):
            if isinstance(inst, mybir.InstLdweights):
                key = str(inst.ins[0])
                if key == last_key:
                    drop.append(idx)
                last_key = key
        for idx in reversed(drop):
            inst = insts[idx]
            si = inst.sync_info
            if si is not None and (len(si.on_wait) or len(si.on_update)):
                nxt = insts[idx + 1]
                nsi = nxt.sync_info
                if nsi is None:
                    nxt.sync_info = si
                else:
                    nsi.on_wait.extend(si.on_wait)
                    nsi.on_update.extend(si.on_update)
            del insts[idx]


def _emit(nc, tc, q_in, k_in, kw1t_d, kw2t_d, qw1t_d, qw2t_d, qw3t_d,
          kb1_d, kb2_d, qb1_d, qb2_d, qb3_d, attn_out, logp_out):
    from contextlib import ExitStack
    ctx = ExitStack()
    with ctx:
        singles = ctx.enter_context(tc.tile_pool(name="singles", bufs=1))
        p_in = ctx.enter_context(tc.tile_pool(name="p_in", bufs=2))
        p_k = ctx.enter_context(tc.tile_pool(name="p_k", bufs=2))
        p_q = ctx.enter_context(tc.tile_pool(name="p_q", bufs=2))
        p_soft = ctx.enter_context(tc.tile_pool(name="p_soft", bufs=4))
        p_small = ctx.enter_context(tc.tile_pool(name="p_small", bufs=8))
        p_att = ctx.enter_context(tc.tile_pool(name="p_att", bufs=2))
        p_lgp = ctx.enter_context(tc.tile_pool(name="p_lgp", bufs=2))
        pp_conv = ctx.enter_context(
            tc.tile_pool(name="pp_conv", bufs=2, space="PSUM"))
        pp_score = ctx.enter_context(
            tc.tile_pool(name="pp_score", bufs=2, space="PSUM"))

        nc.scalar.add_instruction(mybir.InstLoadActFuncSet(
            name=nc.get_next_instruction_name(), ins=[], outs=[],
            act_func_set_id=ACT_SET_ALL))

        # ---------------- weights / constants into SBUF (once) -------------
        w_kw1 = singles.tile([128, 4, 3, DH], DT_K)
        for c in range(4):
            nc.sync.dma_start(out=w_kw1[:, c], in_=kw1t_d[c])
        w_kw2 = singles.tile([128, 2, 3, DH], DT_K1)
        for c in range(2):
            nc.gpsimd.dma_start(out=w_kw2[:, c], in_=kw2t_d[c])
        w_qw1 = singles.tile([128, 8, DH], DT_Q)
        nc.gpsimd.dma_start(out=w_qw1[:D_DEC], in_=qw1t_d[:])
        w_qw2 = singles.tile([128, 2, 7, DH], DT_Q1)
        for c in range(2):
            nc.gpsimd.dma_start(out=w_qw2[:, c], in_=qw2t_d[c])
        w_qw3 = singles.tile([128, 2, 7, DH], DT_Q2)
        for c in range(2):
            nc.gpsimd.dma_start(out=w_qw3[:, c], in_=qw3t_d[c])

        b_k1 = singles.tile([128, 2], F32)
        b_k2 = singles.tile([128, 2], F32)
        b_q1 = singles.tile([128, 2], F32)
        b_q2 = singles.tile([128, 2], F32)
        b_q3 = singles.tile([128, 2], F32)
        for sb_t, dr_t in ((b_k1, kb1_d), (b_k2, kb2_d),
                           (b_q1, qb1_d), (b_q2, qb2_d), (b_q3, qb3_d)):
            for h in range(2):
                nc.gpsimd.dma_start(out=sb_t[:, h:h + 1], in_=dr_t[h])

        ones_col = singles.tile([128, 1], F16)
        nc.vector.memset(ones_col, 1.0)
        ones8 = singles.tile([1, 2, 128], DT8)   # k2 hi/lo DR lhsT (x8 scale)
        nc.vector.memset(ones8, 8.0)
        ones_row = singles.tile([1, 128], F16)
        nc.vector.memset(ones_row, 1.0)

        # persistent padded intermediates; zero the margins once
        k1pad = singles.tile([128, 2, TK1 + 6], DT_K1)
        q1pad = singles.tile([128, 2, TQ + 8], DT_Q1)
        q2pad = singles.tile([128, 2, TQ + 8], DT_Q2)
        for h in range(2):
            nc.vector.memset(k1pad[:, h, 0:3], 0.0)
            nc.vector.memset(k1pad[:, h, TK1 + 3:TK1 + 6], 0.0)
            nc.vector.memset(q1pad[:, h, 0:3], 0.0)
            nc.vector.memset(q1pad[:, h, TQ + 3:TQ + 8], 0.0)
            nc.vector.memset(q2pad[:, h, 0:3], 0.0)
            nc.vector.memset(q2pad[:, h, TQ + 3:TQ + 8], 0.0)

        # software pipeline: batch b's convs interleave with batch b-1's
        # score/softmax tiles so the in-order PE queue never head-blocks on
        # the softmax latency chain
        args = (nc, q_in, k_in, attn_out, logp_out,
                w_kw1, w_kw2, w_qw1, w_qw2, w_qw3,
                b_k1, b_k2, b_q1, b_q2, b_q3,
                ones_col, ones8, ones_row,
                k1pad, q1pad, q2pad,
                p_in, p_k, p_q, p_soft, p_small, p_att, p_lgp,
                pp_conv, pp_score)
        prev_sc = None
        for b in range(BPC):
            out = {}
            cs = _conv_steps(b, out, *args)
            done_c = done_s = False
            while not done_c or not done_s:
                if not done_c:
                    try:
                        next(cs)
                    except StopIteration:
                        done_c = True
                if prev_sc is None:
                    done_s = True
                elif not done_s:
                    for _ in range(2):
                        try:
                            next(prev_sc)
                        except StopIteration:
                            done_s = True
                            break
            prev_sc = _score_steps(b, out, *args)
        for _ in prev_sc:
            pass


def _conv_steps(b, out, nc, q_in, k_in, attn_out, logp_out,
                w_kw1, w_kw2, w_qw1, w_qw2, w_qw3,
                b_k1, b_k2, b_q1, b_q2, b_q3,
                ones_col, ones8, ones_row, k1pad, q1pad, q2pad,
                p_in, p_k, p_q, p_soft, p_small, p_att, p_lgp,
                pp_conv, pp_score):
    mm = nc.tensor.matmul
    act = nc.scalar.activation

    # ---------------- keys path ----------------
    kpad = p_in.tile([128, 4, TK + 6], DT_K, tag="kpad")
    for c in range(4):
        nc.sync.dma_start(out=kpad[:, c, :],
                          in_=k_in[b, 128 * c:128 * (c + 1), :])

    # key conv1: Cin=512, K=3, out [256, 516] -> k1pad (margins pre-zeroed)
    for h in range(2):
        ps = pp_conv.tile([128, 2, 512], F32, tag="conv")
        if KC1_FP8:
            for j in range(3):
                for cp in range(2):
                    for th in range(2):
                        mm(ps[:, th, :HT1],
                           w_kw1[:, 2 * cp:2 * cp + 2, j, 128 * h:128 * (h + 1)],
                           kpad[:, 2 * cp:2 * cp + 2, HT1 * th + j:HT1 * th + j + HT1],
                           start=(j == 0 and cp == 0),
                           stop=(j == 2 and cp == 1), perf_mode=DR)
        else:
            for j in range(3):
                for c in range(4):
                    for th in range(2):
                        mm(ps[:, th, :HT1],
                           w_kw1[:, c, j, 128 * h:128 * (h + 1)],
                           kpad[:, c, HT1 * th + j:HT1 * th + j + HT1],
                           start=(j == 0 and c == 0),
                           stop=(j == 2 and c == 3))
        act(k1pad[:, h, 3:3 + TK1].rearrange("p (a b) -> p a b", a=2),
            ps[:, :, :HT1], AF.Prelu, bias=b_k1[:, h:h + 1], scale=1.0,
            alpha=SLOPE)
        yield

    # key conv2: Cin=256, K=3, out [256, 520] -> ksb (f16, for the scores)
    ksb = p_k.tile([128, 2, TK2], F16, tag="ksb")
    for h in range(2):
        ps = pp_conv.tile([128, 2, 512], F32, tag="conv")
        if KC2_FP8:
            for j in range(3):
                for sh in range(2):
                    mm(ps[:, sh, :HT2],
                       w_kw2[:, 0:2, j, 128 * h:128 * (h + 1)],
                       k1pad[:, 0:2, HT2 * sh + j:HT2 * sh + j + HT2],
                       start=(j == 0), stop=(j == 2), perf_mode=DR)
        else:
            for j in range(3):
                for c in range(2):
                    for sh in range(2):
                        mm(ps[:, sh, :HT2],
                           w_kw2[:, c, j, 128 * h:128 * (h + 1)],
                           k1pad[:, c, HT2 * sh + j:HT2 * sh + j + HT2],
                           start=(j == 0 and c == 0),
                           stop=(j == 2 and c == 1))
        act(ksb[:, h, :].rearrange("p (a b) -> p a b", a=2),
            ps[:, :, :HT2], AF.Prelu, bias=b_k2[:, h:h + 1], scale=1.0,
            alpha=SLOPE)
        yield

    # k2[s] = sum_c k[c,s]^2; k2hl holds fp8 hi/lo of -0.5*k2/8
    veng = nc.vector
    ksq = p_k.tile([128, 2, TK2], F16, tag="ksq")
    veng.tensor_tensor(out=ksq[:, :, :], in0=ksb[:, :, :],
                       in1=ksb[:, :, :], op=ALU.mult)
    ps2 = pp_score.tile([128, 2, 512], F32, tag="sc")
    for c in range(2):
        for sh in range(2):
            mm(ps2[0:1, sh, :HT2], ones_col[:, :],
               ksq[:, c, HT2 * sh:HT2 * sh + HT2],
               start=(c == 0), stop=(c == 1))
    if K2_DR:
        k2hl = p_k.tile([1, 2, TK2], DT8, tag="k2hl")
        k2v = p_k.tile([1, TK2], F16, tag="k2v")
        act(k2hl[:, 0, :].rearrange("p (a b) -> p a b", a=2), ps2[0:1, :, :HT2],
            AF.Copy, bias=0.0, scale=-1.0 / 16.0)
        act(k2v[:, :].rearrange("p (a b) -> p a b", a=2), ps2[0:1, :, :HT2],
            AF.Copy, bias=0.0, scale=-1.0 / 16.0)
        veng.tensor_tensor(out=k2hl[:, 1, :], in0=k2v[:, :],
                           in1=k2hl[:, 0, :], op=ALU.subtract)
        k2row = None
    else:
        k2hl = None
        k2row = p_k.tile([1, TK2], F16, tag="k2row")
        act(k2row[:, :].rearrange("p (a b) -> p a b", a=2), ps2[0:1, :, :HT2],
            AF.Copy, bias=0.0, scale=-0.5)
    out["ksb"], out["k2hl"], out["k2row"] = ksb, k2hl, k2row
    yield

    # ---------------- queries path ----------------
    qpad = p_in.tile([128, 2, TQ + 8], DT_Q, tag="qpad")
    nc.sync.dma_start(out=qpad[:D_DEC, :, :], in_=q_in[b])

    # query conv1: Cin=80, K=7 (padded to 8 taps), tap-pair DoubleRow
    for h in range(2):
        pss = [pp_conv.tile([128, 2, 512], F32, tag="conv", name=f"q1_{b}_{h}_{g}")
               for g in range(2)]
        if QC1_FP8:
            for jp in range(4):
                for g in range(2):
                    for i in range(2):
                        t4 = 2 * g + i
                        mm(pss[g][:, i, :],
                           w_qw1[:D_DEC, 2 * jp:2 * jp + 2, 128 * h:128 * (h + 1)],
                           qpad[:D_DEC, 0:2, 512 * t4 + 2 * jp:512 * t4 + 2 * jp + 512],
                           start=(jp == 0), stop=(jp == 3), perf_mode=DR)
        else:
            for j in range(7):
                for g in range(2):
                    for i in range(2):
                        t4 = 2 * g + i
                        mm(pss[g][:, i, :],
                           w_qw1[:D_DEC, j, 128 * h:128 * (h + 1)],
                           qpad[:D_DEC, 0, 512 * t4 + j:512 * t4 + j + 512],
                           start=(j == 0), stop=(j == 6))
        for g in range(2):
            act(q1pad[:, h, 3 + 1024 * g:3 + 1024 * (g + 1)]
                .rearrange("p (a b) -> p a b", a=2), pss[g][:, :, :],
                AF.Prelu, bias=b_q1[:, h:h + 1], scale=1.0, alpha=SLOPE)
        yield

    # query conv2: Cin=256, K=7, channel-pair DoubleRow, j-outer
    for h in range(2):
        pss = [pp_conv.tile([128, 2, 512], F32, tag="conv",
                            name=f"q2_{b}_{h}_{g}") for g in range(2)]
        if QC2_FP8:
            for j in range(7):
                for g in range(2):
                    for i in range(2):
                        t4 = 2 * g + i
                        mm(pss[g][:, i, :],
                           w_qw2[:, 0:2, j, 128 * h:128 * (h + 1)],
                           q1pad[:, 0:2, 512 * t4 + j:512 * t4 + j + 512],
                           start=(j == 0), stop=(j == 6), perf_mode=DR)
        else:
            for j in range(7):
                for c in range(2):
                    for g in range(2):
                        for i in range(2):
                            t4 = 2 * g + i
                            mm(pss[g][:, i, :],
                               w_qw2[:, c, j, 128 * h:128 * (h + 1)],
                               q1pad[:, c, 512 * t4 + j:512 * t4 + j + 512],
                               start=(j == 0 and c == 0),
                               stop=(j == 6 and c == 1))
        for g in range(2):
            act(q2pad[:, h, 3 + 1024 * g:3 + 1024 * (g + 1)]
                .rearrange("p (a b) -> p a b", a=2), pss[g][:, :, :],
                AF.Prelu, bias=b_q2[:, h:h + 1], scale=1.0, alpha=SLOPE)
        yield

    # query conv3 -> q3 (f16, for the scores); prelu on DVE, bias on PE
    q3 = p_q.tile([128, 2, TQ], F16, tag="q3")
    for h in range(2):
        pss = [pp_conv.tile([128, 2, 512], F32, tag="conv",
                            name=f"q3_{b}_{h}_{g}") for g in range(2)]
        if QC3_FP8:
            for j in range(7):
                for g in range(2):
                    for i in range(2):
                        t4 = 2 * g + i
                        mm(pss[g][:, i, :],
                           w_qw3[:, 0:2, j, 128 * h:128 * (h + 1)],
                           q2pad[:, 0:2, 512 * t4 + j:512 * t4 + j + 512],
                           start=(j == 0), stop=(j == 6), perf_mode=DR)
        else:
            for j in range(7):
                for c in range(2):
                    for g in range(2):
                        for i in range(2):
                            t4 = 2 * g + i
                            mm(pss[g][:, i, :],
                               w_qw3[:, c, j, 128 * h:128 * (h + 1)],
                               q2pad[:, c, 512 * t4 + j:512 * t4 + j + 512],
                               start=(j == 0 and c == 0),
                               stop=(j == 6 and c == 1))
        for g in range(2):
            act(q3[:, h, 1024 * g:1024 * (g + 1)]
                .rearrange("p (a b) -> p a b", a=2), pss[g][:, :, :],
                AF.Prelu, bias=b_q3[:, h:h + 1], scale=1.0, alpha=SLOPE)
        yield
    out["q3"] = q3


def _score_steps(b, out, nc, q_in, k_in, attn_out, logp_out,
                 w_kw1, w_kw2, w_qw1, w_qw2, w_qw3,
                 b_k1, b_k2, b_q1, b_q2, b_q3,
                 ones_col, ones8, ones_row, k1pad, q1pad, q2pad,
                 p_in, p_k, p_q, p_soft, p_small, p_att, p_lgp,
                 pp_conv, pp_score):
    mm = nc.tensor.matmul
    act = nc.scalar.activation
    ksb, k2hl, k2row, q3 = out["ksb"], out["k2hl"], out["k2row"], out["q3"]
    attn_g = logp_g = None
    for t in range(TQ // 128):
        g, i = divmod(t, 2)
        pool = pp_conv if (b == BPC - 1 and t % 2 == 1) else pp_score
        sp = pool.tile([128, 2, 512], F32, tag="sc" if pool is pp_score else "conv",
                       name=f"sp{b}_{t}")
        spf = sp.rearrange("p a b -> p (a b)")   # [128, 1024] flat, 520 used
        for c in range(2):
            q3w = q3[:, c, 128 * t:128 * (t + 1)]
            mm(spf[:, 0:512], q3w, ksb[:, c, 0:512],
               start=(c == 0), stop=False)
            mm(spf[:, 512:TK2], q3w, ksb[:, c, 512:TK2],
               start=(c == 0), stop=False)
        if K2_DR:
            mm(spf[:, 0:512], ones8[:, :, :], k2hl[:, :, 0:512],
               start=False, stop=True, perf_mode=DR)
            mm(spf[:, 512:TK2], ones8[:, :, :], k2hl[:, :, 512:TK2],
               start=False, stop=True, perf_mode=DR)
        else:
            mm(spf[:, 0:512], ones_row[:, :], k2row[:, 0:512],
               start=False, stop=True)
            mm(spf[:, 512:TK2], ones_row[:, :], k2row[:, 512:TK2],
               start=False, stop=True)

        esb = p_soft.tile([128, TK2], F16, tag="esb", name=f"esb{b}_{t}")
        z = p_small.tile([128, 1], F32, tag="z")
        act(esb[:, :], spf[:, 0:TK2], AF.Exp, bias=0.0, scale=SC,
            accum_out=z)
        if i == 0:
            attn_g = p_att.tile([128, 2, TK2], F16, tag="attn")
            logp_g = p_lgp.tile([128, 2, TK2], F16, tag="logp")
        lnz = p_small.tile([128, 1], F32, tag="lnz")
        act(lnz, z, AF.Ln)
        rz = p_small.tile([128, 1], F32, tag="rz")
        nc.vector.reciprocal(rz, z)
        nc.vector.tensor_scalar(attn_g[:, i, :], esb[:, :], rz, None, ALU.mult)
        nc.vector.tensor_scalar(logp_g[:, i, :], spf[:, 0:TK2],
                                SC, lnz, ALU.mult, ALU.subtract)
        if i == 1:
            if OUT_GROUP:
                dst_a = attn_out[b, 256 * g:256 * (g + 1), :] \
                    .rearrange("(a p) s -> p a s", a=2)
                dst_l = logp_out[b, 256 * g:256 * (g + 1), :] \
                    .rearrange("(a p) s -> p a s", a=2)
                nc.sync.dma_start(out=dst_a, in_=attn_g[:, :, :])
                deng = nc.gpsimd if GPS_OPS else nc.sync
                deng.dma_start(out=dst_l, in_=logp_g[:, :, :])
            else:
                deng = nc.gpsimd if GPS_OPS else nc.sync
                for ii in range(2):
                    tt = 2 * g + ii
                    nc.sync.dma_start(
                        out=attn_out[b, 128 * tt:128 * (tt + 1), :],
                        in_=attn_g[:, ii, :])
                    deng.dma_start(
                        out=logp_out[b, 128 * tt:128 * (tt + 1), :],
                        in_=logp_g[:, ii, :])
        yield


_PROGRAM = None


def _get_program():
    global _PROGRAM
    if _PROGRAM is None:
        _PROGRAM = build_program()
    return _PROGRAM


def prep_inputs(queries, keys, kw1, kb1, kw2, kb2, qw1, qb1, qw2, qb2, qw3, qb3):
    """Build the 8 per-core input maps from full-size inputs."""
    f16 = np.float16
    f32 = np.float32
    n_k, n_k1 = mybir.dt.np(DT_K), mybir.dt.np(DT_K1)
    n_q, n_q1, n_q2 = mybir.dt.np(DT_Q), mybir.dt.np(DT_Q1), mybir.dt.np(DT_Q2)

    kw1t = np.ascontiguousarray(
        np.transpose(kw1, (1, 2, 0)).reshape(4, 128, 3, DH).astype(n_k))
    kw2t = np.ascontiguousarray(
        np.transpose(kw2, (1, 2, 0)).reshape(2, 128, 3, DH).astype(n_k1))
    qw1t = np.zeros((D_DEC, 8, DH), n_q)
    qw1t[:, :7, :] = np.transpose(qw1, (1, 2, 0))
    qw2t = np.ascontiguousarray(
        np.transpose(qw2, (1, 2, 0)).reshape(2, 128, 7, DH).astype(n_q1))
    qw3t = np.ascontiguousarray(
        np.transpose(qw3, (1, 2, 0)).reshape(2, 128, 7, DH).astype(n_q2))
    shared = dict(
        kw1t=kw1t, kw2t=kw2t, qw1t=qw1t, qw2t=qw2t, qw3t=qw3t,
        kb1c=np.ascontiguousarray(kb1.reshape(2, 128, 1), f32),
        kb2c=np.ascontiguousarray(kb2.reshape(2, 128, 1), f32),
        qb1c=np.ascontiguousarray(qb1.reshape(2, 128, 1), f32),
        qb2c=np.ascontiguousarray(qb2.reshape(2, 128, 1), f32),
        qb3c=np.ascontiguousarray(qb3.reshape(2, 128, 1), f32),
    )
    B = queries.shape[0]
    qp = np.zeros((B, D_DEC, 2, TQ + 8), n_q)
    qp[:, :, 0, 3:TQ + 3] = queries
    qp[:, :, 1, 2:TQ + 2] = queries
    kp = np.zeros((B, D_ENC, TK + 6), n_k)
    kp[:, :, 3:TK + 3] = keys
    in_maps = []
    for i in range(N_CORES):
        m = dict(shared)
        m["queries"] = np.ascontiguousarray(qp[BPC * i:BPC * (i + 1)])
        m["keys"] = np.ascontiguousarray(kp[BPC * i:BPC * (i + 1)])
        in_maps.append(m)
    return in_maps


def run(in_maps, **kwargs):
    nc = _get_program()
    return run_bass_kernel_spmd(nc, in_maps, core_ids=list(range(N_CORES)), **kwargs)


def kernel(queries, keys, kw1, kb1, kw2, kb2, qw1, qb1, qw2, qb2, qw3, qb3,
           **kwargs):
    in_maps = prep_inputs(queries, keys, kw1, kb1, kw2, kb2,
                          qw1, qb1, qw2, qb2, qw3, qb3)
    res = run(in_maps)
    attn = np.concatenate([np.asarray(r["attn_out"], np.float32)
                           for r in res.results], axis=0)
    logp = np.concatenate([np.asarray(r["logp_out"], np.float32)
                           for r in res.results], axis=0)
    B = attn.shape[0]
    return attn.reshape(B, 1, TQ, TK2), logp.reshape(B, 1, TQ, TK2)


# revision 9
# speedup vs baseline: 1.1532x; 1.1532x over previous
"""Trainium2 Bass kernel for the Aligner2 problem (v2: fp8 DoubleRow).

Computes, for each batch b:
  k = LReLU(conv3(LReLU(conv3(keys))))                   # [256, 520]
  q = LReLU(conv7(LReLU(conv7(LReLU(conv7(queries))))))  # [256, 2048]
  sp[t,s] = sum_c q[c,t] k[c,s] - 0.5*k2[s]              # PE
  l = 2*TEMP*sp  (the -TEMP*q2 term cancels in log_softmax)
  attn = exp(l)/z;  logp = l - ln(z);  z = sum_s exp(l)

Precision: query convs in fp8e4m3 with DoubleRow (two 128-deep
contraction subtiles per pass), key convs and scores in f16, the
-0.5*k2 rank-1 row as an fp8 hi/lo DoubleRow pair, outputs f16
(upcast to f32 on host).

Engine split (GpSimd cannot touch PSUM on TRN2): Exp/lnz and the
qc1/qc2 leaky-ReLUs on Scalar (bias via the act bias port); kc1/kc2/qc3
leaky-ReLUs plus z/rz/logp on DVE (their biases enter the PSUM group as
rank-1 bias x ones matmuls); ksq / attn scaling on GpSimd (SBUF only).
Data-parallel over batch: 4 batches/core, 8 cores.
"""
import numpy as np

import concourse.bass as bass
import concourse.bacc as bacc
import concourse.tile as tile
from concourse import mybir
from concourse.bass_utils import run_bass_kernel_spmd

F32 = mybir.dt.float32
F16 = mybir.dt.float16
DT8 = mybir.dt.float8e4
AF = mybir.ActivationFunctionType
ALU = mybir.AluOpType
DR = mybir.MatmulPerfMode.DoubleRow

SLOPE = 0.3
TEMPERATURE = 0.0005
SC = 2.0 * TEMPERATURE

BPC = 4          # batches per core
N_CORES = 8
D_DEC, TQ = 80, 2048
D_ENC, TK = 512, 512
DH = 256
TK1 = TK + 4     # 516 after key conv1 (kernel 3, pad 3)
TK2 = TK + 8     # 520 after key conv2
HT1 = TK1 // 2   # 258
HT2 = TK2 // 2   # 260

import os
def _flag(name, default):
    return os.environ.get(name, default) == "1"

# per-layer matmul precision: fp8e4m3 DoubleRow vs plain f16
KC1_FP8 = _flag("KV2_KC1", "0")
KC2_FP8 = _flag("KV2_KC2", "0")
QC1_FP8 = _flag("KV2_QC1", "1")
QC2_FP8 = _flag("KV2_QC2", "1")
QC3_FP8 = _flag("KV2_QC3", "1")
K2_DR = _flag("KV2_K2DR", "1")      # fp8 hi/lo DoubleRow k2 row vs f16 single
GPS_OPS = _flag("KV2_GPS", "1")     # gpsimd for ksq/k2lo/attn + logp dma
OUT_GROUP = _flag("KV2_OUTG", "1")  # grouped strided out-DMA vs per-tile
DVE3_PRELU = _flag("KV2_DVE3", "1")  # qc3 prelu on DVE (else Scalar)

# remove back-to-back LDWEIGHTS with identical weight APs before compile
DEDUP_LDW = _flag("KV2_DEDUP", "0")

ACT_SET_ALL = 6  # act_info.json set containing Prelu/Exp/Ln/Copy together

DT_K = DT8 if KC1_FP8 else F16    # kpad, kw1t
DT_K1 = DT8 if KC2_FP8 else F16   # k1pad, kw2t
DT_Q = DT8 if QC1_FP8 else F16    # qpad, qw1t
DT_Q1 = DT8 if QC2_FP8 else F16   # q1pad, qw2t
DT_Q2 = DT8 if QC3_FP8 else F16   # q2pad, qw3t


def build_program():
    nc = bacc.Bacc("TRN2", target_bir_lowering=False)

    # two planes: plane 1 is pre-shifted left by one sample on the host so
    # tap-pair DoubleRow reads are plain strided slices (overlapping APs
    # crash walrus codegen)
    q_in = nc.dram_tensor("queries", [BPC, D_DEC, 2, TQ + 8], DT_Q, kind="ExternalInput")
    k_in = nc.dram_tensor("keys", [BPC, D_ENC, TK + 6], DT_K, kind="ExternalInput")
    kw1t_d = nc.dram_tensor("kw1t", [4, 128, 3, DH], DT_K, kind="ExternalInput")
    kw2t_d = nc.dram_tensor("kw2t", [2, 128, 3, DH], DT_K1, kind="ExternalInput")
    qw1t_d = nc.dram_tensor("qw1t", [D_DEC, 8, DH], DT_Q, kind="ExternalInput")
    qw2t_d = nc.dram_tensor("qw2t", [2, 128, 7, DH], DT_Q1, kind="ExternalInput")
    qw3t_d = nc.dram_tensor("qw3t", [2, 128, 7, DH], DT_Q2, kind="ExternalInput")
    # row-format biases feed rank-1 bias matmuls (DVE-prelu layers);
    # col-format biases feed the act bias port (Scalar-prelu layers)
    kb1_d = nc.dram_tensor("kb1c", [2, 128, 1], F32, kind="ExternalInput")
    kb2_d = nc.dram_tensor("kb2c", [2, 128, 1], F32, kind="ExternalInput")
    qb1_d = nc.dram_tensor("qb1c", [2, 128, 1], F32, kind="ExternalInput")
    qb2_d = nc.dram_tensor("qb2c", [2, 128, 1], F32, kind="ExternalInput")
    qb3_d = nc.dram_tensor("qb3c", [2, 128, 1], F32, kind="ExternalInput")
    attn_out = nc.dram_tensor("attn_out", [BPC, TQ, TK2], F16, kind="ExternalOutput")
    logp_out = nc.dram_tensor("logp_out", [BPC, TQ, TK2], F16, kind="ExternalOutput")

    with tile.TileContext(nc) as tc:
        _emit(nc, tc, q_in, k_in, kw1t_d, kw2t_d, qw1t_d, qw2t_d, qw3t_d,
              kb1_d, kb2_d, qb1_d, qb2_d, qb3_d, attn_out, logp_out)
    if DEDUP_LDW:
        _dedup_ldweights(nc)
    nc.compile()
    return nc


def _dedup_ldweights(nc):
    """Drop an InstLdweights whose weights AP is identical to the previous
    InstLdweights in the same block (nothing between them invalidates the
    PE-resident weights). Its waits/updates move to the following matmul;
    multi-wait legalization runs later in compile."""
    for bb in nc.main_func.blocks:
        insts = bb.instructions
        last_key = None
        drop = []
        for idx, inst in enumerate(insts):
            if isinstance(inst, mybir.InstLdweights):
                key = str(inst.ins[0])
                if key == last_key:
                    drop.append(idx)
                last_key = key
        for idx in reversed(drop):
            inst = insts[idx]
            si = inst.sync_info
            if si is not None and (len(si.on_wait) or len(si.on_update)):
                nxt = insts[idx + 1]
                nsi = nxt.sync_info
                if nsi is None:
                    nxt.sync_info = si
                else:
                    nsi.on_wait.extend(si.on_wait)
                    nsi.on_update.extend(si.on_update)
            del insts[idx]


def _emit(nc, tc, q_in, k_in, kw1t_d, kw2t_d, qw1t_d, qw2t_d, qw3t_d,
          kb1_d, kb2_d, qb1_d, qb2_d, qb3_d, attn_out, logp_out):
    from contextlib import ExitStack
    ctx = ExitStack()
    with ctx:
        singles = ctx.enter_context(tc.tile_pool(name="singles", bufs=1))
        p_in = ctx.enter_context(tc.tile_pool(name="p_in", bufs=2))
        p_k = ctx.enter_context(tc.tile_pool(name="p_k", bufs=2))
        p_q = ctx.enter_context(tc.tile_pool(name="p_q", bufs=2))
        p_soft = ctx.enter_context(tc.tile_pool(name="p_soft", bufs=4))
        p_small = ctx.enter_context(tc.tile_pool(name="p_small", bufs=8))
        p_att = ctx.enter_context(tc.tile_pool(name="p_att", bufs=2))
        p_lgp = ctx.enter_context(tc.tile_pool(name="p_lgp", bufs=2))
        pp_conv = ctx.enter_context(
            tc.tile_pool(name="pp_conv", bufs=2, space="PSUM"))
        pp_score = ctx.enter_context(
            tc.tile_pool(name="pp_score", bufs=2, space="PSUM"))

        nc.scalar.add_instruction(mybir.InstLoadActFuncSet(
            name=nc.get_next_instruction_name(), ins=[], outs=[],
            act_func_set_id=ACT_SET_ALL))

        # ---------------- weights / constants into SBUF (once) -------------
        w_kw1 = singles.tile([128, 4, 3, DH], DT_K)
        for c in range(4):
            nc.sync.dma_start(out=w_kw1[:, c], in_=kw1t_d[c])
        w_kw2 = singles.tile([128, 2, 3, DH], DT_K1)
        for c in range(2):
            nc.sync.dma_start(out=w_kw2[:, c], in_=kw2t_d[c])
        w_qw1 = singles.tile([128, 8, DH], DT_Q)
        nc.sync.dma_start(out=w_qw1[:D_DEC], in_=qw1t_d[:])
        w_qw2 = singles.tile([128, 2, 7, DH], DT_Q1)
        for c in range(2):
            nc.sync.dma_start(out=w_qw2[:, c], in_=qw2t_d[c])
        w_qw3 = singles.tile([128, 2, 7, DH], DT_Q2)
        for c in range(2):
            nc.sync.dma_start(out=w_qw3[:, c], in_=qw3t_d[c])

        b_k1 = singles.tile([128, 2], F32)
        b_k2 = singles.tile([128, 2], F32)
        b_q1 = singles.tile([128, 2], F32)
        b_q2 = singles.tile([128, 2], F32)
        b_q3 = singles.tile([128, 2], F32)
        for sb_t, dr_t in ((b_k1, kb1_d), (b_k2, kb2_d),
                           (b_q1, qb1_d), (b_q2, qb2_d), (b_q3, qb3_d)):
            for h in range(2):
                nc.sync.dma_start(out=sb_t[:, h:h + 1], in_=dr_t[h])

        ones_col = singles.tile([128, 1], F16)
        nc.vector.memset(ones_col, 1.0)
        ones8 = singles.tile([1, 2, 128], DT8)   # k2 hi/lo DR lhsT (x8 scale)
        nc.vector.memset(ones8, 8.0)
        ones_row = singles.tile([1, 128], F16)
        nc.vector.memset(ones_row, 1.0)

        # persistent padded intermediates; zero the margins once
        k1pad = singles.tile([128, 2, TK1 + 6], DT_K1)
        q1pad = singles.tile([128, 2, TQ + 8], DT_Q1)
        q2pad = singles.tile([128, 2, TQ + 8], DT_Q2)
        for h in range(2):
            nc.vector.memset(k1pad[:, h, 0:3], 0.0)
            nc.vector.memset(k1pad[:, h, TK1 + 3:TK1 + 6], 0.0)
            nc.vector.memset(q1pad[:, h, 0:3], 0.0)
            nc.vector.memset(q1pad[:, h, TQ + 3:TQ + 8], 0.0)
            nc.vector.memset(q2pad[:, h, 0:3], 0.0)
            nc.vector.memset(q2pad[:, h, TQ + 3:TQ + 8], 0.0)

        # software pipeline: batch b's convs interleave with batch b-1's
        # score/softmax tiles so the in-order PE queue never head-blocks on
        # the softmax latency chain
        args = (nc, q_in, k_in, attn_out, logp_out,
                w_kw1, w_kw2, w_qw1, w_qw2, w_qw3,
                b_k1, b_k2, b_q1, b_q2, b_q3,
                ones_col, ones8, ones_row,
                k1pad, q1pad, q2pad,
                p_in, p_k, p_q, p_soft, p_small, p_att, p_lgp,
                pp_conv, pp_score)
        prev_sc = None
        for b in range(BPC):
            out = {}
            cs = _conv_steps(b, out, *args)
            done_c = done_s = False
            while not done_c or not done_s:
                if not done_c:
                    try:
                        next(cs)
                    except StopIteration:
                        done_c = True
                if prev_sc is None:
                    done_s = True
                elif not done_s:
                    for _ in range(2):
                        try:
                            next(prev_sc)
                        except StopIteration:
                            done_s = True
                            break
            prev_sc = _score_steps(b, out, *args)
        for _ in prev_sc:
            pass


def _conv_steps(b, out, nc, q_in, k_in, attn_out, logp_out,
                w_kw1, w_kw2, w_qw1, w_qw2, w_qw3,
                b_k1, b_k2, b_q1, b_q2, b_q3,
                ones_col, ones8, ones_row, k1pad, q1pad, q2pad,
                p_in, p_k, p_q, p_soft, p_small, p_att, p_lgp,
                pp_conv, pp_score):
    mm = nc.tensor.matmul
    act = nc.scalar.activation

    # ---------------- keys path ----------------
    kpad = p_in.tile([128, 4, TK + 6], DT_K, tag="kpad")
    for c in range(4):
        nc.sync.dma_start(out=kpad[:, c, :],
                          in_=k_in[b, 128 * c:128 * (c + 1), :])

    # key conv1: Cin=512, K=3, out [256, 516] -> k1pad (margins pre-zeroed)
    for h in range(2):
        ps = pp_conv.tile([128, 2, 512], F32, tag="conv")
        if KC1_FP8:
            for j in range(3):
                for cp in range(2):
                    for th in range(2):
                        mm(ps[:, th, :HT1],
                           w_kw1[:, 2 * cp:2 * cp + 2, j, 128 * h:128 * (h + 1)],
                           kpad[:, 2 * cp:2 * cp + 2, HT1 * th + j:HT1 * th + j + HT1],
                           start=(j == 0 and cp == 0),
                           stop=(j == 2 and cp == 1), perf_mode=DR)
        else:
            for j in range(3):
                for c in range(4):
                    for th in range(2):
                        mm(ps[:, th, :HT1],
                           w_kw1[:, c, j, 128 * h:128 * (h + 1)],
                           kpad[:, c, HT1 * th + j:HT1 * th + j + HT1],
                           start=(j == 0 and c == 0),
                           stop=(j == 2 and c == 3))
        act(k1pad[:, h, 3:3 + TK1].rearrange("p (a b) -> p a b", a=2),
            ps[:, :, :HT1], AF.Prelu, bias=b_k1[:, h:h + 1], scale=1.0,
            alpha=SLOPE)
        yield

    # key conv2: Cin=256, K=3, out [256, 520] -> ksb (f16, for the scores)
    ksb = p_k.tile([128, 2, TK2], F16, tag="ksb")
    for h in range(2):
        ps = pp_conv.tile([128, 2, 512], F32, tag="conv")
        if KC2_FP8:
            for j in range(3):
                for sh in range(2):
                    mm(ps[:, sh, :HT2],
                       w_kw2[:, 0:2, j, 128 * h:128 * (h + 1)],
                       k1pad[:, 0:2, HT2 * sh + j:HT2 * sh + j + HT2],
                       start=(j == 0), stop=(j == 2), perf_mode=DR)
        else:
            for j in range(3):
                for c in range(2):
                    for sh in range(2):
                        mm(ps[:, sh, :HT2],
                           w_kw2[:, c, j, 128 * h:128 * (h + 1)],
                           k1pad[:, c, HT2 * sh + j:HT2 * sh + j + HT2],
                           start=(j == 0 and c == 0),
                           stop=(j == 2 and c == 1))
        act(ksb[:, h, :].rearrange("p (a b) -> p a b", a=2),
            ps[:, :, :HT2], AF.Prelu, bias=b_k2[:, h:h + 1], scale=1.0,
            alpha=SLOPE)
        yield

    # k2[s] = sum_c k[c,s]^2; k2hl holds fp8 hi/lo of -0.5*k2/8
    veng = nc.vector
    ksq = p_k.tile([128, 2, TK2], F16, tag="ksq")
    veng.tensor_tensor(out=ksq[:, :, :], in0=ksb[:, :, :],
                       in1=ksb[:, :, :], op=ALU.mult)
    ps2 = pp_score.tile([128, 2, 512], F32, tag="sc")
    for c in range(2):
        for sh in range(2):
            mm(ps2[0:1, sh, :HT2], ones_col[:, :],
               ksq[:, c, HT2 * sh:HT2 * sh + HT2],
               start=(c == 0), stop=(c == 1))
    if K2_DR:
        k2hl = p_k.tile([1, 2, TK2], DT8, tag="k2hl")
        k2v = p_k.tile([1, TK2], F16, tag="k2v")
        act(k2hl[:, 0, :].rearrange("p (a b) -> p a b", a=2), ps2[0:1, :, :HT2],
            AF.Copy, bias=0.0, scale=-1.0 / 16.0)
        act(k2v[:, :].rearrange("p (a b) -> p a b", a=2), ps2[0:1, :, :HT2],
            AF.Copy, bias=0.0, scale=-1.0 / 16.0)
        veng.tensor_tensor(out=k2hl[:, 1, :], in0=k2v[:, :],
                           in1=k2hl[:, 0, :], op=ALU.subtract)
        k2row = None
    else:
        k2hl = None
        k2row = p_k.tile([1, TK2], F16, tag="k2row")
        act(k2row[:, :].rearrange("p (a b) -> p a b", a=2), ps2[0:1, :, :HT2],
            AF.Copy, bias=0.0, scale=-0.5)
    out["ksb"], out["k2hl"], out["k2row"] = ksb, k2hl, k2row
    yield

    # ---------------- queries path ----------------
    qpad = p_in.tile([128, 2, TQ + 8], DT_Q, tag="qpad")
    nc.sync.dma_start(out=qpad[:D_DEC, :, :], in_=q_in[b])

    # query conv1: Cin=80, K=7 (padded to 8 taps), tap-pair DoubleRow
    for h in range(2):
        pss = [pp_conv.tile([128, 2, 512], F32, tag="conv", name=f"q1_{b}_{h}_{g}")
               for g in range(2)]
        if QC1_FP8:
            for jp in range(4):
                for g in range(2):
                    for i in range(2):
                        t4 = 2 * g + i
                        mm(pss[g][:, i, :],
                           w_qw1[:D_DEC, 2 * jp:2 * jp + 2, 128 * h:128 * (h + 1)],
                           qpad[:D_DEC, 0:2, 512 * t4 + 2 * jp:512 * t4 + 2 * jp + 512],
                           start=(jp == 0), stop=(jp == 3), perf_mode=DR)
        else:
            for j in range(7):
                for g in range(2):
                    for i in range(2):
                        t4 = 2 * g + i
                        mm(pss[g][:, i, :],
                           w_qw1[:D_DEC, j, 128 * h:128 * (h + 1)],
                           qpad[:D_DEC, 0, 512 * t4 + j:512 * t4 + j + 512],
                           start=(j == 0), stop=(j == 6))
        for g in range(2):
            act(q1pad[:, h, 3 + 1024 * g:3 + 1024 * (g + 1)]
                .rearrange("p (a b) -> p a b", a=2), pss[g][:, :, :],
                AF.Prelu, bias=b_q1[:, h:h + 1], scale=1.0, alpha=SLOPE)
        yield

    # query conv2: Cin=256, K=7, channel-pair DoubleRow, j-outer
    for h in range(2):
        pss = [pp_conv.tile([128, 2, 512], F32, tag="conv",
                            name=f"q2_{b}_{h}_{g}") for g in range(2)]
        if QC2_FP8:
            for j in range(7):
                for g in range(2):
                    for i in range(2):
                        t4 = 2 * g + i
                        mm(pss[g][:, i, :],
                           w_qw2[:, 0:2, j, 128 * h:128 * (h + 1)],
                           q1pad[:, 0:2, 512 * t4 + j:512 * t4 + j + 512],
                           start=(j == 0), stop=(j == 6), perf_mode=DR)
        else:
            for j in range(7):
                for c in range(2):
                    for g in range(2):
                        for i in range(2):
                            t4 = 2 * g + i
                            mm(pss[g][:, i, :],
                               w_qw2[:, c, j, 128 * h:128 * (h + 1)],
                               q1pad[:, c, 512 * t4 + j:512 * t4 + j + 512],
                               start=(j == 0 and c == 0),
                               stop=(j == 6 and c == 1))
        for g in range(2):
            act(q2pad[:, h, 3 + 1024 * g:3 + 1024 * (g + 1)]
                .rearrange("p (a b) -> p a b", a=2), pss[g][:, :, :],
                AF.Prelu, bias=b_q2[:, h:h + 1], scale=1.0, alpha=SLOPE)
        yield

    # query conv3 -> q3 (f16, for the scores); prelu on DVE, bias on PE
    q3 = p_q.tile([128, 2, TQ], F16, tag="q3")
    for h in range(2):
        pss = [pp_conv.tile([128, 2, 512], F32, tag="conv",
                            name=f"q3_{b}_{h}_{g}") for g in range(2)]
        if QC3_FP8:
            for j in range(7):
                for g in range(2):
                    for i in range(2):
                        t4 = 2 * g + i
                        mm(pss[g][:, i, :],
                           w_qw3[:, 0:2, j, 128 * h:128 * (h + 1)],
                           q2pad[:, 0:2, 512 * t4 + j:512 * t4 + j + 512],
                           start=(j == 0), stop=(j == 6), perf_mode=DR)
        else:
            for j in range(7):
                for c in range(2):
                    for g in range(2):
                        for i in range(2):
                            t4 = 2 * g + i
                            mm(pss[g][:, i, :],
                               w_qw3[:, c, j, 128 * h:128 * (h + 1)],
                               q2pad[:, c, 512 * t4 + j:512 * t4 + j + 512],
                               start=(j == 0 and c == 0),
                               stop=(j == 6 and c == 1))
        for g in range(2):
            act(q3[:, h, 1024 * g:1024 * (g + 1)]
                .rearrange("p (a b) -> p a b", a=2), pss[g][:, :, :],
                AF.Prelu, bias=b_q3[:, h:h + 1], scale=1.0, alpha=SLOPE)
        yield
    out["q3"] = q3


def _score_steps(b, out, nc, q_in, k_in, attn_out, logp_out,
                 w_kw1, w_kw2, w_qw1, w_qw2, w_qw3,
                 b_k1, b_k2, b_q1, b_q2, b_q3,
                 ones_col, ones8, ones_row, k1pad, q1pad, q2pad,
                 p_in, p_k, p_q, p_soft, p_small, p_att, p_lgp,
                 pp_conv, pp_score):
    mm = nc.tensor.matmul
    act = nc.scalar.activation
    ksb, k2hl, k2row, q3 = out["ksb"], out["k2hl"], out["k2row"], out["q3"]
    attn_g = logp_g = None
    for t in range(TQ // 128):
        g, i = divmod(t, 2)
        sp = pp_score.tile([128, 2, 512], F32, tag="sc", name=f"sp{b}_{t}")
        spf = sp.rearrange("p a b -> p (a b)")   # [128, 1024] flat, 520 used
        for c in range(2):
            q3w = q3[:, c, 128 * t:128 * (t + 1)]
            mm(spf[:, 0:512], q3w, ksb[:, c, 0:512],
               start=(c == 0), stop=False)
            mm(spf[:, 512:TK2], q3w, ksb[:, c, 512:TK2],
               start=(c == 0), stop=False)
        if K2_DR:
            mm(spf[:, 0:512], ones8[:, :, :], k2hl[:, :, 0:512],
               start=False, stop=True, perf_mode=DR)
            mm(spf[:, 512:TK2], ones8[:, :, :], k2hl[:, :, 512:TK2],
               start=False, stop=True, perf_mode=DR)
        else:
            mm(spf[:, 0:512], ones_row[:, :], k2row[:, 0:512],
               start=False, stop=True)
            mm(spf[:, 512:TK2], ones_row[:, :], k2row[:, 512:TK2],
               start=False, stop=True)

        esb = p_soft.tile([128, TK2], F16, tag="esb", name=f"esb{b}_{t}")
        z = p_small.tile([128, 1], F32, tag="z")
        act(esb[:, :], spf[:, 0:TK2], AF.Exp, bias=0.0, scale=SC,
            accum_out=z)
        if i == 0:
            attn_g = p_att.tile([128, 2, TK2], F16, tag="attn")
            logp_g = p_lgp.tile([128, 2, TK2], F16, tag="logp")
        lnz = p_small.tile([128, 1], F32, tag="lnz")
        act(lnz, z, AF.Ln)
        rz = p_small.tile([128, 1], F32, tag="rz")
        nc.vector.reciprocal(rz, z)
        # logp in two steps: the first only reads sp, releasing the PSUM
        # slot without waiting for lnz
        lraw = p_soft.tile([128, TK2], F16, tag="lraw", name=f"lr{b}_{t}")
        nc.vector.tensor_scalar(lraw[:, :], spf[:, 0:TK2], SC, None, ALU.mult)
        nc.vector.tensor_scalar(attn_g[:, i, :], esb[:, :], rz, None, ALU.mult)
        nc.vector.tensor_scalar(logp_g[:, i, :], lraw[:, :], lnz, None,
                                ALU.subtract)
        if i == 1:
            if OUT_GROUP:
                dst_a = attn_out[b, 256 * g:256 * (g + 1), :] \
                    .rearrange("(a p) s -> p a s", a=2)
                dst_l = logp_out[b, 256 * g:256 * (g + 1), :] \
                    .rearrange("(a p) s -> p a s", a=2)
                nc.sync.dma_start(out=dst_a, in_=attn_g[:, :, :])
                deng = nc.gpsimd if GPS_OPS else nc.sync
                deng.dma_start(out=dst_l, in_=logp_g[:, :, :])
            else:
                deng = nc.gpsimd if GPS_OPS else nc.sync
                for ii in range(2):
                    tt = 2 * g + ii
                    nc.sync.dma_start(
                        out=attn_out[b, 128 * tt:128 * (tt + 1), :],
                        in_=attn_g[:, ii, :])
                    deng.dma_start(
                        out=logp_out[b, 128 * tt:128 * (tt + 1), :],
                        in_=logp_g[:, ii, :])
        yield


_PROGRAM = None


def _get_program():
    global _PROGRAM
    if _PROGRAM is None:
        _PROGRAM = build_program()
    return _PROGRAM


def prep_inputs(queries, keys, kw1, kb1, kw2, kb2, qw1, qb1, qw2, qb2, qw3, qb3):
    """Build the 8 per-core input maps from full-size inputs."""
    f16 = np.float16
    f32 = np.float32
    n_k, n_k1 = mybir.dt.np(DT_K), mybir.dt.np(DT_K1)
    n_q, n_q1, n_q2 = mybir.dt.np(DT_Q), mybir.dt.np(DT_Q1), mybir.dt.np(DT_Q2)

    kw1t = np.ascontiguousarray(
        np.transpose(kw1, (1, 2, 0)).reshape(4, 128, 3, DH).astype(n_k))
    kw2t = np.ascontiguousarray(
        np.transpose(kw2, (1, 2, 0)).reshape(2, 128, 3, DH).astype(n_k1))
    qw1t = np.zeros((D_DEC, 8, DH), n_q)
    qw1t[:, :7, :] = np.transpose(qw1, (1, 2, 0))
    qw2t = np.ascontiguousarray(
        np.transpose(qw2, (1, 2, 0)).reshape(2, 128, 7, DH).astype(n_q1))
    qw3t = np.ascontiguousarray(
        np.transpose(qw3, (1, 2, 0)).reshape(2, 128, 7, DH).astype(n_q2))
    shared = dict(
        kw1t=kw1t, kw2t=kw2t, qw1t=qw1t, qw2t=qw2t, qw3t=qw3t,
        kb1c=np.ascontiguousarray(kb1.reshape(2, 128, 1), f32),
        kb2c=np.ascontiguousarray(kb2.reshape(2, 128, 1), f32),
        qb1c=np.ascontiguousarray(qb1.reshape(2, 128, 1), f32),
        qb2c=np.ascontiguousarray(qb2.reshape(2, 128, 1), f32),
        qb3c=np.ascontiguousarray(qb3.reshape(2, 128, 1), f32),
    )
    B = queries.shape[0]
    qp = np.zeros((B, D_DEC, 2, TQ + 8), n_q)
    qp[:, :, 0, 3:TQ + 3] = queries
    qp[:, :, 1, 2:TQ + 2] = queries
    kp = np.zeros((B, D_ENC, TK + 6), n_k)
    kp[:, :, 3:TK + 3] = keys
    in_maps = []
    for i in range(N_CORES):
        m = dict(shared)
        m["queries"] = np.ascontiguousarray(qp[BPC * i:BPC * (i + 1)])
        m["keys"] = np.ascontiguousarray(kp[BPC * i:BPC * (i + 1)])
        in_maps.append(m)
    return in_maps


def run(in_maps, **kwargs):
    nc = _get_program()
    return run_bass_kernel_spmd(nc, in_maps, core_ids=list(range(N_CORES)), **kwargs)


def kernel(queries, keys, kw1, kb1, kw2, kb2, qw1, qb1, qw2, qb2, qw3, qb3,
           **kwargs):
    in_maps = prep_inputs(queries, keys, kw1, kb1, kw2, kb2,
                          qw1, qb1, qw2, qb2, qw3, qb3)
    res = run(in_maps)
    attn = np.concatenate([np.asarray(r["attn_out"], np.float32)
                           for r in res.results], axis=0)
    logp = np.concatenate([np.asarray(r["logp_out"], np.float32)
                           for r in res.results], axis=0)
    B = attn.shape[0]
    return attn.reshape(B, 1, TQ, TK2), logp.reshape(B, 1, TQ, TK2)


# revision 10
# speedup vs baseline: 1.1578x; 1.0040x over previous
"""Trainium2 Bass kernel for the Aligner2 problem (v2: fp8 DoubleRow).

Computes, for each batch b:
  k = LReLU(conv3(LReLU(conv3(keys))))                   # [256, 520]
  q = LReLU(conv7(LReLU(conv7(LReLU(conv7(queries))))))  # [256, 2048]
  sp[t,s] = sum_c q[c,t] k[c,s] - 0.5*k2[s]              # PE
  l = 2*TEMP*sp  (the -TEMP*q2 term cancels in log_softmax)
  attn = exp(l)/z;  logp = l - ln(z);  z = sum_s exp(l)

Precision: query convs in fp8e4m3 with DoubleRow (two 128-deep
contraction subtiles per pass), key convs and scores in f16, the
-0.5*k2 rank-1 row as an fp8 hi/lo DoubleRow pair, outputs f16
(upcast to f32 on host).

Engine split (GpSimd cannot touch PSUM on TRN2): Exp/lnz and the
qc1/qc2 leaky-ReLUs on Scalar (bias via the act bias port); kc1/kc2/qc3
leaky-ReLUs plus z/rz/logp on DVE (their biases enter the PSUM group as
rank-1 bias x ones matmuls); ksq / attn scaling on GpSimd (SBUF only).
Data-parallel over batch: 4 batches/core, 8 cores.
"""
import numpy as np

import concourse.bass as bass
import concourse.bacc as bacc
import concourse.tile as tile
from concourse import mybir
from concourse.bass_utils import run_bass_kernel_spmd

F32 = mybir.dt.float32
F16 = mybir.dt.float16
DT8 = mybir.dt.float8e4
AF = mybir.ActivationFunctionType
ALU = mybir.AluOpType
DR = mybir.MatmulPerfMode.DoubleRow

SLOPE = 0.3
TEMPERATURE = 0.0005
SC = 2.0 * TEMPERATURE

BPC = 4          # batches per core
N_CORES = 8
D_DEC, TQ = 80, 2048
D_ENC, TK = 512, 512
DH = 256
TK1 = TK + 4     # 516 after key conv1 (kernel 3, pad 3)
TK2 = TK + 8     # 520 after key conv2
HT1 = TK1 // 2   # 258
HT2 = TK2 // 2   # 260

import os
def _flag(name, default):
    return os.environ.get(name, default) == "1"

# per-layer matmul precision: fp8e4m3 DoubleRow vs plain f16
KC1_FP8 = _flag("KV2_KC1", "0")
KC2_FP8 = _flag("KV2_KC2", "0")
QC1_FP8 = _flag("KV2_QC1", "1")
QC2_FP8 = _flag("KV2_QC2", "1")
QC3_FP8 = _flag("KV2_QC3", "1")
K2_DR = _flag("KV2_K2DR", "1")      # fp8 hi/lo DoubleRow k2 row vs f16 single
GPS_OPS = _flag("KV2_GPS", "1")     # gpsimd for ksq/k2lo/attn + logp dma
OUT_GROUP = _flag("KV2_OUTG", "1")  # grouped strided out-DMA vs per-tile
DVE3_PRELU = _flag("KV2_DVE3", "1")  # qc3 prelu on DVE (else Scalar)

# remove back-to-back LDWEIGHTS with identical weight APs before compile
DEDUP_LDW = _flag("KV2_DEDUP", "0")

ACT_SET_ALL = 6  # act_info.json set containing Prelu/Exp/Ln/Copy together

DT_K = DT8 if KC1_FP8 else F16    # kpad, kw1t
DT_K1 = DT8 if KC2_FP8 else F16   # k1pad, kw2t
DT_Q = DT8 if QC1_FP8 else F16    # qpad, qw1t
DT_Q1 = DT8 if QC2_FP8 else F16   # q1pad, qw2t
DT_Q2 = DT8 if QC3_FP8 else F16   # q2pad, qw3t


def build_program():
    nc = bacc.Bacc("TRN2", target_bir_lowering=False)

    # two planes: plane 1 is pre-shifted left by one sample on the host so
    # tap-pair DoubleRow reads are plain strided slices (overlapping APs
    # crash walrus codegen)
    q_in = nc.dram_tensor("queries", [BPC, D_DEC, 2, TQ + 8], DT_Q, kind="ExternalInput")
    k_in = nc.dram_tensor("keys", [BPC, D_ENC, TK + 6], DT_K, kind="ExternalInput")
    kw1t_d = nc.dram_tensor("kw1t", [4, 128, 3, DH], DT_K, kind="ExternalInput")
    kw2t_d = nc.dram_tensor("kw2t", [2, 128, 3, DH], DT_K1, kind="ExternalInput")
    qw1t_d = nc.dram_tensor("qw1t", [D_DEC, 8, DH], DT_Q, kind="ExternalInput")
    qw2t_d = nc.dram_tensor("qw2t", [2, 128, 7, DH], DT_Q1, kind="ExternalInput")
    qw3t_d = nc.dram_tensor("qw3t", [2, 128, 7, DH], DT_Q2, kind="ExternalInput")
    # row-format biases feed rank-1 bias matmuls (DVE-prelu layers);
    # col-format biases feed the act bias port (Scalar-prelu layers)
    kb1_d = nc.dram_tensor("kb1c", [2, 128, 1], F32, kind="ExternalInput")
    kb2_d = nc.dram_tensor("kb2c", [2, 128, 1], F32, kind="ExternalInput")
    qb1_d = nc.dram_tensor("qb1c", [2, 128, 1], F32, kind="ExternalInput")
    qb2_d = nc.dram_tensor("qb2c", [2, 128, 1], F32, kind="ExternalInput")
    qb3_d = nc.dram_tensor("qb3c", [2, 128, 1], F32, kind="ExternalInput")
    attn_out = nc.dram_tensor("attn_out", [BPC, TQ, TK2], F16, kind="ExternalOutput")
    logp_out = nc.dram_tensor("logp_out", [BPC, TQ, TK2], F16, kind="ExternalOutput")

    with tile.TileContext(nc) as tc:
        _emit(nc, tc, q_in, k_in, kw1t_d, kw2t_d, qw1t_d, qw2t_d, qw3t_d,
              kb1_d, kb2_d, qb1_d, qb2_d, qb3_d, attn_out, logp_out)
    if DEDUP_LDW:
        _dedup_ldweights(nc)
    nc.compile()
    return nc


def _dedup_ldweights(nc):
    """Drop an InstLdweights whose weights AP is identical to the previous
    InstLdweights in the same block (nothing between them invalidates the
    PE-resident weights). Its waits/updates move to the following matmul;
    multi-wait legalization runs later in compile."""
    for bb in nc.main_func.blocks:
        insts = bb.instructions
        last_key = None
        drop = []
        for idx, inst in enumerate(insts):
            if isinstance(inst, mybir.InstLdweights):
                key = str(inst.ins[0])
                if key == last_key:
                    drop.append(idx)
                last_key = key
        for idx in reversed(drop):
            inst = insts[idx]
            si = inst.sync_info
            if si is not None and (len(si.on_wait) or len(si.on_update)):
                nxt = insts[idx + 1]
                nsi = nxt.sync_info
                if nsi is None:
                    nxt.sync_info = si
                else:
                    nsi.on_wait.extend(si.on_wait)
                    nsi.on_update.extend(si.on_update)
            del insts[idx]


def _emit(nc, tc, q_in, k_in, kw1t_d, kw2t_d, qw1t_d, qw2t_d, qw3t_d,
          kb1_d, kb2_d, qb1_d, qb2_d, qb3_d, attn_out, logp_out):
    from contextlib import ExitStack
    ctx = ExitStack()
    with ctx:
        singles = ctx.enter_context(tc.tile_pool(name="singles", bufs=1))
        p_in = ctx.enter_context(tc.tile_pool(name="p_in", bufs=2))
        p_k = ctx.enter_context(tc.tile_pool(name="p_k", bufs=2))
        p_q = ctx.enter_context(tc.tile_pool(name="p_q", bufs=2))
        p_soft = ctx.enter_context(tc.tile_pool(name="p_soft", bufs=4))
        p_small = ctx.enter_context(tc.tile_pool(name="p_small", bufs=8))
        p_att = ctx.enter_context(tc.tile_pool(name="p_att", bufs=2))
        p_lgp = ctx.enter_context(tc.tile_pool(name="p_lgp", bufs=2))
        pp_conv = ctx.enter_context(
            tc.tile_pool(name="pp_conv", bufs=2, space="PSUM"))
        pp_score = ctx.enter_context(
            tc.tile_pool(name="pp_score", bufs=2, space="PSUM"))

        nc.scalar.add_instruction(mybir.InstLoadActFuncSet(
            name=nc.get_next_instruction_name(), ins=[], outs=[],
            act_func_set_id=ACT_SET_ALL))

        # ---------------- weights / constants into SBUF (once) -------------
        w_kw1 = singles.tile([128, 4, 3, DH], DT_K)
        for c in range(4):
            nc.sync.dma_start(out=w_kw1[:, c], in_=kw1t_d[c])
        w_kw2 = singles.tile([128, 2, 3, DH], DT_K1)
        for c in range(2):
            nc.sync.dma_start(out=w_kw2[:, c], in_=kw2t_d[c])
        w_qw1 = singles.tile([128, 8, DH], DT_Q)
        nc.sync.dma_start(out=w_qw1[:D_DEC], in_=qw1t_d[:])
        w_qw2 = singles.tile([128, 2, 7, DH], DT_Q1)
        for c in range(2):
            nc.sync.dma_start(out=w_qw2[:, c], in_=qw2t_d[c])
        w_qw3 = singles.tile([128, 2, 7, DH], DT_Q2)
        for c in range(2):
            nc.sync.dma_start(out=w_qw3[:, c], in_=qw3t_d[c])

        b_k1 = singles.tile([128, 2], F32)
        b_k2 = singles.tile([128, 2], F32)
        b_q1 = singles.tile([128, 2], F32)
        b_q2 = singles.tile([128, 2], F32)
        b_q3 = singles.tile([128, 2], F32)
        for sb_t, dr_t in ((b_k1, kb1_d), (b_k2, kb2_d),
                           (b_q1, qb1_d), (b_q2, qb2_d), (b_q3, qb3_d)):
            for h in range(2):
                nc.sync.dma_start(out=sb_t[:, h:h + 1], in_=dr_t[h])

        ones_col = singles.tile([128, 1], F16)
        nc.vector.memset(ones_col, 1.0)
        ones8 = singles.tile([1, 2, 128], DT8)   # k2 hi/lo DR lhsT (x8 scale)
        nc.vector.memset(ones8, 8.0)
        ones_row = singles.tile([1, 128], F16)
        nc.vector.memset(ones_row, 1.0)

        # persistent padded intermediates; zero the margins once
        k1pad = singles.tile([128, 2, TK1 + 6], DT_K1)
        q1pad = singles.tile([128, 2, TQ + 8], DT_Q1)
        q2pad = singles.tile([128, 2, TQ + 8], DT_Q2)
        for h in range(2):
            nc.vector.memset(k1pad[:, h, 0:3], 0.0)
            nc.vector.memset(k1pad[:, h, TK1 + 3:TK1 + 6], 0.0)
            nc.vector.memset(q1pad[:, h, 0:3], 0.0)
            nc.vector.memset(q1pad[:, h, TQ + 3:TQ + 8], 0.0)
            nc.vector.memset(q2pad[:, h, 0:3], 0.0)
            nc.vector.memset(q2pad[:, h, TQ + 3:TQ + 8], 0.0)

        # software pipeline: batch b's convs interleave with batch b-1's
        # score/softmax tiles so the in-order PE queue never head-blocks on
        # the softmax latency chain
        args = (nc, q_in, k_in, attn_out, logp_out,
                w_kw1, w_kw2, w_qw1, w_qw2, w_qw3,
                b_k1, b_k2, b_q1, b_q2, b_q3,
                ones_col, ones8, ones_row,
                k1pad, q1pad, q2pad,
                p_in, p_k, p_q, p_soft, p_small, p_att, p_lgp,
                pp_conv, pp_score)
        prev_sc = None
        for b in range(BPC):
            out = {}
            cs = _conv_steps(b, out, *args)
            done_c = done_s = False
            while not done_c or not done_s:
                if not done_c:
                    try:
                        next(cs)
                    except StopIteration:
                        done_c = True
                if prev_sc is None:
                    done_s = True
                elif not done_s:
                    for _ in range(2):
                        try:
                            next(prev_sc)
                        except StopIteration:
                            done_s = True
                            break
            prev_sc = _score_steps(b, out, *args)
        for _ in prev_sc:
            pass


def _conv_steps(b, out, nc, q_in, k_in, attn_out, logp_out,
                w_kw1, w_kw2, w_qw1, w_qw2, w_qw3,
                b_k1, b_k2, b_q1, b_q2, b_q3,
                ones_col, ones8, ones_row, k1pad, q1pad, q2pad,
                p_in, p_k, p_q, p_soft, p_small, p_att, p_lgp,
                pp_conv, pp_score):
    mm = nc.tensor.matmul
    act = nc.scalar.activation

    # ---------------- keys path ----------------
    kpad = p_in.tile([128, 4, TK + 6], DT_K, tag="kpad")
    for c in range(4):
        nc.sync.dma_start(out=kpad[:, c, :],
                          in_=k_in[b, 128 * c:128 * (c + 1), :])

    # key conv1: Cin=512, K=3, out [256, 516] -> k1pad (margins pre-zeroed)
    for h in range(2):
        ps = pp_conv.tile([128, 2, 512], F32, tag="conv")
        if KC1_FP8:
            for j in range(3):
                for cp in range(2):
                    for th in range(2):
                        mm(ps[:, th, :HT1],
                           w_kw1[:, 2 * cp:2 * cp + 2, j, 128 * h:128 * (h + 1)],
                           kpad[:, 2 * cp:2 * cp + 2, HT1 * th + j:HT1 * th + j + HT1],
                           start=(j == 0 and cp == 0),
                           stop=(j == 2 and cp == 1), perf_mode=DR)
        else:
            for j in range(3):
                for c in range(4):
                    for th in range(2):
                        mm(ps[:, th, :HT1],
                           w_kw1[:, c, j, 128 * h:128 * (h + 1)],
                           kpad[:, c, HT1 * th + j:HT1 * th + j + HT1],
                           start=(j == 0 and c == 0),
                           stop=(j == 2 and c == 3))
        act(k1pad[:, h, 3:3 + TK1].rearrange("p (a b) -> p a b", a=2),
            ps[:, :, :HT1], AF.Prelu, bias=b_k1[:, h:h + 1], scale=1.0,
            alpha=SLOPE)
        yield

    # key conv2: Cin=256, K=3, out [256, 520] -> ksb (f16, for the scores)
    ksb = p_k.tile([128, 2, TK2], F16, tag="ksb")
    for h in range(2):
        ps = pp_conv.tile([128, 2, 512], F32, tag="conv")
        if KC2_FP8:
            for j in range(3):
                for sh in range(2):
                    mm(ps[:, sh, :HT2],
                       w_kw2[:, 0:2, j, 128 * h:128 * (h + 1)],
                       k1pad[:, 0:2, HT2 * sh + j:HT2 * sh + j + HT2],
                       start=(j == 0), stop=(j == 2), perf_mode=DR)
        else:
            for j in range(3):
                for c in range(2):
                    for sh in range(2):
                        mm(ps[:, sh, :HT2],
                           w_kw2[:, c, j, 128 * h:128 * (h + 1)],
                           k1pad[:, c, HT2 * sh + j:HT2 * sh + j + HT2],
                           start=(j == 0 and c == 0),
                           stop=(j == 2 and c == 1))
        act(ksb[:, h, :].rearrange("p (a b) -> p a b", a=2),
            ps[:, :, :HT2], AF.Prelu, bias=b_k2[:, h:h + 1], scale=1.0,
            alpha=SLOPE)
        yield

    # k2[s] = sum_c k[c,s]^2; k2hl holds fp8 hi/lo of -0.5*k2/8
    veng = nc.vector
    ksq = p_k.tile([128, 2, TK2], F16, tag="ksq")
    veng.tensor_tensor(out=ksq[:, :, :], in0=ksb[:, :, :],
                       in1=ksb[:, :, :], op=ALU.mult)
    ps2 = pp_score.tile([128, 2, 512], F32, tag="sc")
    for c in range(2):
        for sh in range(2):
            mm(ps2[0:1, sh, :HT2], ones_col[:, :],
               ksq[:, c, HT2 * sh:HT2 * sh + HT2],
               start=(c == 0), stop=(c == 1))
    if K2_DR:
        k2hl = p_k.tile([1, 2, TK2], DT8, tag="k2hl")
        k2v = p_k.tile([1, TK2], F16, tag="k2v")
        act(k2hl[:, 0, :].rearrange("p (a b) -> p a b", a=2), ps2[0:1, :, :HT2],
            AF.Copy, bias=0.0, scale=-1.0 / 16.0)
        act(k2v[:, :].rearrange("p (a b) -> p a b", a=2), ps2[0:1, :, :HT2],
            AF.Copy, bias=0.0, scale=-1.0 / 16.0)
        veng.tensor_tensor(out=k2hl[:, 1, :], in0=k2v[:, :],
                           in1=k2hl[:, 0, :], op=ALU.subtract)
        k2row = None
    else:
        k2hl = None
        k2row = p_k.tile([1, TK2], F16, tag="k2row")
        act(k2row[:, :].rearrange("p (a b) -> p a b", a=2), ps2[0:1, :, :HT2],
            AF.Copy, bias=0.0, scale=-0.5)
    out["ksb"], out["k2hl"], out["k2row"] = ksb, k2hl, k2row
    yield

    # ---------------- queries path ----------------
    qpad = p_in.tile([128, 2, TQ + 8], DT_Q, tag="qpad")
    nc.sync.dma_start(out=qpad[:D_DEC, :, :], in_=q_in[b])

    # query conv1: Cin=80, K=7 (padded to 8 taps), tap-pair DoubleRow
    for h in range(2):
        pss = [pp_conv.tile([128, 2, 512], F32, tag="conv", name=f"q1_{b}_{h}_{g}")
               for g in range(2)]
        if QC1_FP8:
            for jp in range(4):
                for g in range(2):
                    for i in range(2):
                        t4 = 2 * g + i
                        mm(pss[g][:, i, :],
                           w_qw1[:D_DEC, 2 * jp:2 * jp + 2, 128 * h:128 * (h + 1)],
                           qpad[:D_DEC, 0:2, 512 * t4 + 2 * jp:512 * t4 + 2 * jp + 512],
                           start=(jp == 0), stop=(jp == 3), perf_mode=DR)
        else:
            for j in range(7):
                for g in range(2):
                    for i in range(2):
                        t4 = 2 * g + i
                        mm(pss[g][:, i, :],
                           w_qw1[:D_DEC, j, 128 * h:128 * (h + 1)],
                           qpad[:D_DEC, 0, 512 * t4 + j:512 * t4 + j + 512],
                           start=(j == 0), stop=(j == 6))
        for g in range(2):
            act(q1pad[:, h, 3 + 1024 * g:3 + 1024 * (g + 1)]
                .rearrange("p (a b) -> p a b", a=2), pss[g][:, :, :],
                AF.Prelu, bias=b_q1[:, h:h + 1], scale=1.0, alpha=SLOPE)
        yield

    # query conv2: Cin=256, K=7, channel-pair DoubleRow, j-outer
    for h in range(2):
        pss = [pp_conv.tile([128, 2, 512], F32, tag="conv",
                            name=f"q2_{b}_{h}_{g}") for g in range(2)]
        if QC2_FP8:
            for j in range(7):
                for g in range(2):
                    for i in range(2):
                        t4 = 2 * g + i
                        mm(pss[g][:, i, :],
                           w_qw2[:, 0:2, j, 128 * h:128 * (h + 1)],
                           q1pad[:, 0:2, 512 * t4 + j:512 * t4 + j + 512],
                           start=(j == 0), stop=(j == 6), perf_mode=DR)
        else:
            for j in range(7):
                for c in range(2):
                    for g in range(2):
                        for i in range(2):
                            t4 = 2 * g + i
                            mm(pss[g][:, i, :],
                               w_qw2[:, c, j, 128 * h:128 * (h + 1)],
                               q1pad[:, c, 512 * t4 + j:512 * t4 + j + 512],
                               start=(j == 0 and c == 0),
                               stop=(j == 6 and c == 1))
        for g in range(2):
            act(q2pad[:, h, 3 + 1024 * g:3 + 1024 * (g + 1)]
                .rearrange("p (a b) -> p a b", a=2), pss[g][:, :, :],
                AF.Prelu, bias=b_q2[:, h:h + 1], scale=1.0, alpha=SLOPE)
        yield

    # query conv3 -> q3 (f16, for the scores); prelu on DVE, bias on PE
    q3 = p_q.tile([128, 2, TQ], F16, tag="q3")
    for h in range(2):
        pss = [pp_conv.tile([128, 2, 512], F32, tag="conv",
                            name=f"q3_{b}_{h}_{g}") for g in range(2)]
        if QC3_FP8:
            for j in range(7):
                for g in range(2):
                    for i in range(2):
                        t4 = 2 * g + i
                        mm(pss[g][:, i, :],
                           w_qw3[:, 0:2, j, 128 * h:128 * (h + 1)],
                           q2pad[:, 0:2, 512 * t4 + j:512 * t4 + j + 512],
                           start=(j == 0), stop=(j == 6), perf_mode=DR)
        else:
            for j in range(7):
                for c in range(2):
                    for g in range(2):
                        for i in range(2):
                            t4 = 2 * g + i
                            mm(pss[g][:, i, :],
                               w_qw3[:, c, j, 128 * h:128 * (h + 1)],
                               q2pad[:, c, 512 * t4 + j:512 * t4 + j + 512],
                               start=(j == 0 and c == 0),
                               stop=(j == 6 and c == 1))
        for g in range(2):
            act(q3[:, h, 1024 * g:1024 * (g + 1)]
                .rearrange("p (a b) -> p a b", a=2), pss[g][:, :, :],
                AF.Prelu, bias=b_q3[:, h:h + 1], scale=1.0, alpha=SLOPE)
        yield
    out["q3"] = q3


def _score_steps(b, out, nc, q_in, k_in, attn_out, logp_out,
                 w_kw1, w_kw2, w_qw1, w_qw2, w_qw3,
                 b_k1, b_k2, b_q1, b_q2, b_q3,
                 ones_col, ones8, ones_row, k1pad, q1pad, q2pad,
                 p_in, p_k, p_q, p_soft, p_small, p_att, p_lgp,
                 pp_conv, pp_score):
    mm = nc.tensor.matmul
    act = nc.scalar.activation
    ksb, k2hl, k2row, q3 = out["ksb"], out["k2hl"], out["k2row"], out["q3"]
    attn_g = logp_g = None
    for t in range(TQ // 128):
        g, i = divmod(t, 2)
        pool = pp_conv if (b == BPC - 1 and t % 2 == 1) else pp_score
        sp = pool.tile([128, 2, 512], F32,
                       tag="sc" if pool is pp_score else "conv",
                       name=f"sp{b}_{t}")
        spf = sp.rearrange("p a b -> p (a b)")   # [128, 1024] flat, 520 used
        for c in range(2):
            q3w = q3[:, c, 128 * t:128 * (t + 1)]
            mm(spf[:, 0:512], q3w, ksb[:, c, 0:512],
               start=(c == 0), stop=False)
            mm(spf[:, 512:TK2], q3w, ksb[:, c, 512:TK2],
               start=(c == 0), stop=False)
        if K2_DR:
            mm(spf[:, 0:512], ones8[:, :, :], k2hl[:, :, 0:512],
               start=False, stop=True, perf_mode=DR)
            mm(spf[:, 512:TK2], ones8[:, :, :], k2hl[:, :, 512:TK2],
               start=False, stop=True, perf_mode=DR)
        else:
            mm(spf[:, 0:512], ones_row[:, :], k2row[:, 0:512],
               start=False, stop=True)
            mm(spf[:, 512:TK2], ones_row[:, :], k2row[:, 512:TK2],
               start=False, stop=True)

        esb = p_soft.tile([128, TK2], F16, tag="esb", name=f"esb{b}_{t}")
        z = p_small.tile([128, 1], F32, tag="z")
        act(esb[:, :], spf[:, 0:TK2], AF.Exp, bias=0.0, scale=SC,
            accum_out=z)
        if i == 0:
            attn_g = p_att.tile([128, 2, TK2], F16, tag="attn")
            logp_g = p_lgp.tile([128, 2, TK2], F16, tag="logp")
        lnz = p_small.tile([128, 1], F32, tag="lnz")
        act(lnz, z, AF.Ln)
        rz = p_small.tile([128, 1], F32, tag="rz")
        nc.vector.reciprocal(rz, z)
        # logp in two steps: the first only reads sp, releasing the PSUM
        # slot without waiting for lnz
        lraw = p_soft.tile([128, TK2], F16, tag="lraw", name=f"lr{b}_{t}")
        nc.vector.tensor_scalar(lraw[:, :], spf[:, 0:TK2], SC, None, ALU.mult)
        nc.vector.tensor_scalar(attn_g[:, i, :], esb[:, :], rz, None, ALU.mult)
        nc.vector.tensor_scalar(logp_g[:, i, :], lraw[:, :], lnz, None,
                                ALU.subtract)
        if i == 1:
            if OUT_GROUP:
                dst_a = attn_out[b, 256 * g:256 * (g + 1), :] \
                    .rearrange("(a p) s -> p a s", a=2)
                dst_l = logp_out[b, 256 * g:256 * (g + 1), :] \
                    .rearrange("(a p) s -> p a s", a=2)
                nc.sync.dma_start(out=dst_a, in_=attn_g[:, :, :])
                deng = nc.gpsimd if GPS_OPS else nc.sync
                deng.dma_start(out=dst_l, in_=logp_g[:, :, :])
            else:
                deng = nc.gpsimd if GPS_OPS else nc.sync
                for ii in range(2):
                    tt = 2 * g + ii
                    nc.sync.dma_start(
                        out=attn_out[b, 128 * tt:128 * (tt + 1), :],
                        in_=attn_g[:, ii, :])
                    deng.dma_start(
                        out=logp_out[b, 128 * tt:128 * (tt + 1), :],
                        in_=logp_g[:, ii, :])
        yield


_PROGRAM = None


def _get_program():
    global _PROGRAM
    if _PROGRAM is None:
        _PROGRAM = build_program()
    return _PROGRAM


def prep_inputs(queries, keys, kw1, kb1, kw2, kb2, qw1, qb1, qw2, qb2, qw3, qb3):
    """Build the 8 per-core input maps from full-size inputs."""
    f16 = np.float16
    f32 = np.float32
    n_k, n_k1 = mybir.dt.np(DT_K), mybir.dt.np(DT_K1)
    n_q, n_q1, n_q2 = mybir.dt.np(DT_Q), mybir.dt.np(DT_Q1), mybir.dt.np(DT_Q2)

    kw1t = np.ascontiguousarray(
        np.transpose(kw1, (1, 2, 0)).reshape(4, 128, 3, DH).astype(n_k))
    kw2t = np.ascontiguousarray(
        np.transpose(kw2, (1, 2, 0)).reshape(2, 128, 3, DH).astype(n_k1))
    qw1t = np.zeros((D_DEC, 8, DH), n_q)
    qw1t[:, :7, :] = np.transpose(qw1, (1, 2, 0))
    qw2t = np.ascontiguousarray(
        np.transpose(qw2, (1, 2, 0)).reshape(2, 128, 7, DH).astype(n_q1))
    qw3t = np.ascontiguousarray(
        np.transpose(qw3, (1, 2, 0)).reshape(2, 128, 7, DH).astype(n_q2))
    shared = dict(
        kw1t=kw1t, kw2t=kw2t, qw1t=qw1t, qw2t=qw2t, qw3t=qw3t,
        kb1c=np.ascontiguousarray(kb1.reshape(2, 128, 1), f32),
        kb2c=np.ascontiguousarray(kb2.reshape(2, 128, 1), f32),
        qb1c=np.ascontiguousarray(qb1.reshape(2, 128, 1), f32),
        qb2c=np.ascontiguousarray(qb2.reshape(2, 128, 1), f32),
        qb3c=np.ascontiguousarray(qb3.reshape(2, 128, 1), f32),
    )
    B = queries.shape[0]
    qp = np.zeros((B, D_DEC, 2, TQ + 8), n_q)
    qp[:, :, 0, 3:TQ + 3] = queries
    qp[:, :, 1, 2:TQ + 2] = queries
    kp = np.zeros((B, D_ENC, TK + 6), n_k)
    kp[:, :, 3:TK + 3] = keys
    in_maps = []
    for i in range(N_CORES):
        m = dict(shared)
        m["queries"] = np.ascontiguousarray(qp[BPC * i:BPC * (i + 1)])
        m["keys"] = np.ascontiguousarray(kp[BPC * i:BPC * (i + 1)])
        in_maps.append(m)
    return in_maps


def run(in_maps, **kwargs):
    nc = _get_program()
    return run_bass_kernel_spmd(nc, in_maps, core_ids=list(range(N_CORES)), **kwargs)


def kernel(queries, keys, kw1, kb1, kw2, kb2, qw1, qb1, qw2, qb2, qw3, qb3,
           **kwargs):
    in_maps = prep_inputs(queries, keys, kw1, kb1, kw2, kb2,
                          qw1, qb1, qw2, qb2, qw3, qb3)
    res = run(in_maps)
    attn = np.concatenate([np.asarray(r["attn_out"], np.float32)
                           for r in res.results], axis=0)
    logp = np.concatenate([np.asarray(r["logp_out"], np.float32)
                           for r in res.results], axis=0)
    B = attn.shape[0]
    return attn.reshape(B, 1, TQ, TK2), logp.reshape(B, 1, TQ, TK2)
